# revision 18
# baseline (speedup 1.0000x reference)
"""Multi-head attention on 8 TRN2 NeuronCores.

Sharding: 4-way data-parallel over batch x 2-way tensor-parallel over heads.
Core c handles batch (c // 2) and heads [8*(c%2), 8*(c%2)+8).

Per-core kernel (feature-major / transposed layouts throughout):
  xT   [1024, 2048]  (bf16, d-major)           -> SBUF
  Q^T, K^T = Wq/Wk slices @ xT + bias           [512, 2048] (e-major, bf16)
             (1/sqrt(dk) folded into Wq, bq on host)
  V    = x @ Wv^T slice + bias, token-major     [2048, 8 heads, 64+1]
         (65th column = ones -> softmax denominator comes free in ctx matmul)
  S^T[k,q] = K^T.T @ Q^T per head               (two heads packed in the
             128-row PE array via tile_position row groups, contraction=64)
  P = exp(S^T)  (scores are small: |S|<~3, so no max-subtraction needed)
  ctx^T[d,q] (+denom row) = V.T @ P             accumulated over 16 k-tiles
  ctx^T normalized by 1/denom (partition_broadcast + DVE multiply)
  outT_partial[e,t] = Wo^T slice.T @ ctx^T      [1024, 2048] f32 -> DRAM

Host: out[b] = (outT_core(2b) + outT_core(2b+1)).T + bo.
"""

import numpy as np
import ml_dtypes
from contextlib import ExitStack

import concourse.bass as bass
import concourse.bacc as bacc
import concourse.mybir as mybir
import concourse.tile as tile
from concourse.bass_utils import run_bass_kernel_spmd

D = 1024          # d_model
HEADS = 16
DK = 64           # head dim
B = 4             # batch
S = 2048          # sequence length
TP = 2            # tensor-parallel ways (over heads)
DP = 4            # data-parallel ways (over batch)
N_CORES = 8
EL = D // TP      # 512 local projection dims
HL = HEADS // TP  # 8 local heads
T = S             # tokens per core (one batch)
KT = D // 128     # 8 contraction tiles for projections
TT = T // 128     # 16 token tiles
NQ = T // 512     # 4 query tiles
NK = T // 128     # 16 key tiles

F32 = mybir.dt.float32
BF16 = mybir.dt.bfloat16
AF = mybir.ActivationFunctionType
ALU = mybir.AluOpType


def _bcast_ap(ap: bass.AP, parts: int) -> bass.AP:
    """Prepend a step-0 partition dim: broadcast a (partition-less or 1-row)
    AP across `parts` partitions for DMA."""
    return bass.AP(tensor=ap.tensor, offset=ap.offset, ap=[[0, parts]] + list(ap.ap))


def build_program() -> bass.Bass:
    nc = bacc.Bacc("TRN2", debug=False)

    xT = nc.dram_tensor("xT", [D, T], BF16, kind="ExternalInput").ap()
    wqT = nc.dram_tensor("wqT", [D, EL], BF16, kind="ExternalInput").ap()
    wkT = nc.dram_tensor("wkT", [D, EL], BF16, kind="ExternalInput").ap()
    wvT = nc.dram_tensor("wvT", [D, EL], BF16, kind="ExternalInput").ap()
    woT = nc.dram_tensor("woT", [EL, D], BF16, kind="ExternalInput").ap()
    bq = nc.dram_tensor("bq", [EL], BF16, kind="ExternalInput").ap()
    bk = nc.dram_tensor("bk", [EL], BF16, kind="ExternalInput").ap()
    bv = nc.dram_tensor("bv", [EL], BF16, kind="ExternalInput").ap()
    outT = nc.dram_tensor("outT", [D, T], F32, kind="ExternalOutput").ap()

    with ExitStack() as ctx:
        tc = ctx.enter_context(tile.TileContext(nc))
        const = ctx.enter_context(tc.tile_pool(name="const", bufs=1))
        xw = ctx.enter_context(tc.tile_pool(name="xw", bufs=1))
        qkv = ctx.enter_context(tc.tile_pool(name="qkv", bufs=1))
        expp = ctx.enter_context(tc.tile_pool(name="expp", bufs=3))
        stage = ctx.enter_context(tc.tile_pool(name="stage", bufs=3))
        psp = ctx.enter_context(tc.tile_pool(name="psp", bufs=2, space="PSUM"))
        ctxp = ctx.enter_context(tc.tile_pool(name="ctxp", bufs=4, space="PSUM"))
        drp = ctx.enter_context(tc.tile_pool(name="drp", bufs=3, space="DRAM"))

        # ---------------- loads ----------------
        xt_sb = xw.tile([128, KT, T], BF16)          # [p, kt, t]
        for kt in range(KT):
            nc.sync.dma_start(out=xt_sb[:, kt, :], in_=xT[kt * 128:(kt + 1) * 128, :])
        wq_sb = xw.tile([128, KT, EL], BF16)
        wk_sb = xw.tile([128, KT, EL], BF16)
        wv_sb = xw.tile([128, KT, EL], BF16)
        for w_sb, w_dram in ((wq_sb, wqT), (wk_sb, wkT), (wv_sb, wvT)):
            for kt in range(KT):
                nc.sync.dma_start(out=w_sb[:, kt, :], in_=w_dram[kt * 128:(kt + 1) * 128, :])
        wo_sb = xw.tile([128, EL // 128, D], BF16)
        for kt in range(EL // 128):
            nc.sync.dma_start(out=wo_sb[:, kt, :], in_=woT[kt * 128:(kt + 1) * 128, :])

        # biases live on one partition row; folded into the matmuls as a
        # K=1 rank-1 update (lhsT/rhs of ones), avoiding extra DVE ops
        bq_sb = const.tile([1, EL], BF16)
        nc.sync.dma_start(out=bq_sb, in_=_bcast_ap(bq, 1))
        bk_sb = const.tile([1, EL], BF16)
        nc.sync.dma_start(out=bk_sb, in_=_bcast_ap(bk, 1))
        bv_sb = const.tile([1, EL], BF16)
        nc.sync.dma_start(out=bv_sb, in_=_bcast_ap(bv, 1))
        ones_sb = const.tile([1, 512], BF16)
        nc.vector.memset(ones_sb, 1.0)

        # ---------------- Q/K/V projections ----------------
        # Order: Q[hp0], K[hp0] first so attention on head-pair 0 (and its
        # exp stream on ScalarE, the bottleneck engine) starts ~15us in,
        # then V (needed by the first ctx matmuls), then remaining Q/K.
        qt_sb = qkv.tile([128, EL // 128, T], BF16)   # [p(=e within hp), hp, t]
        kt_sb = qkv.tile([128, EL // 128, T], BF16)
        v_sb = qkv.tile([128, TT, HL, DK + 1], BF16)  # [p(=t within tt), tt, h, dk+ones]
        nc.vector.memset(v_sb[:, :, :, DK:DK + 1], 1.0)

        def qk_proj(hp):
            for w_sb, b_sb, dst in ((wq_sb, bq_sb, qt_sb), (wk_sb, bk_sb, kt_sb)):
                for nt2 in range(T // 1024):
                    ps = psp.tile([128, 1024], F32, tag="ps", name="ps")
                    for half in range(2):
                        t0 = (nt2 * 2 + half) * 512
                        # bias via rank-1 update: ps = b[e] * ones[t]
                        nc.tensor.matmul(
                            ps[:, half * 512:(half + 1) * 512],
                            lhsT=b_sb[0:1, hp * 128:(hp + 1) * 128],
                            rhs=ones_sb[0:1, :],
                            start=True, stop=False)
                        for kt in range(KT):
                            nc.tensor.matmul(
                                ps[:, half * 512:(half + 1) * 512],
                                lhsT=w_sb[:, kt, hp * 128:(hp + 1) * 128],
                                rhs=xt_sb[:, kt, t0:t0 + 512],
                                start=False, stop=(kt == KT - 1))
                    nc.vector.tensor_copy(
                        out=dst[:, hp, nt2 * 1024:(nt2 + 1) * 1024],
                        in_=ps)

        def v_proj():
            for tt in range(TT):
                psv = psp.tile([128, 512], F32, tag="ps", name="psv")
                # bias via rank-1 update: psv = ones[t] * bv[e]
                nc.tensor.matmul(
                    psv, lhsT=ones_sb[0:1, 0:128], rhs=bv_sb[0:1, :],
                    start=True, stop=False)
                for kt in range(KT):
                    nc.tensor.matmul(
                        psv,
                        lhsT=xt_sb[:, kt, tt * 128:(tt + 1) * 128],
                        rhs=wv_sb[:, kt, :],
                        start=False, stop=(kt == KT - 1))
                nc.vector.tensor_copy(
                    out=v_sb[:, tt, :, 0:DK],
                    in_=psv.rearrange("p (h d) -> p h d", h=HL))

        qk_proj(0)
        v_proj()
        for hp in range(1, EL // 128):
            qk_proj(hp)

        # ---------------- attention + output projection ----------------
        ctxT_sb = qkv.tile([128, EL // 128, T], BF16)  # [p(=d within hp), hp, q]
        outT_r = outT.rearrange("(E p) t -> p E t", p=128)  # [128, 8, 2048]

        def oproj(qt, ets):
            # output projection, one [128, 512] out tile per e-tile
            for et in ets:
                pso = ctxp.tile([128, 512], F32, tag="ctx", name="pso")
                for hp in range(EL // 128):
                    nc.tensor.matmul(
                        pso,
                        lhsT=wo_sb[:, hp, et * 128:(et + 1) * 128],
                        rhs=ctxT_sb[:, hp, qt * 512:(qt + 1) * 512],
                        start=(hp == 0), stop=(hp == EL // 128 - 1))
                ot = stage.tile([128, 512], F32, tag="ot", name="ot")
                nc.vector.tensor_copy(ot, pso)
                nc.sync.dma_start(
                    out=outT_r[:, et, qt * 512:(qt + 1) * 512],
                    in_=ot)

        for qt in range(NQ):
            for hp in range(EL // 128):
                hA, hB = 2 * hp, 2 * hp + 1
                ctxA = ctxp.tile([128, 512], F32, tag="ctx", name="ctxA")
                ctxB = ctxp.tile([128, 512], F32, tag="ctx", name="ctxB")
                for kt in range(NK):
                    ps = psp.tile([128, 1024], F32, tag="ps", name="pss")
                    # S^T tiles for two heads packed into row-groups 0-63 / 64-127
                    nc.tensor.matmul(
                        ps[:, 0:512],
                        lhsT=kt_sb[0:64, hp, kt * 128:(kt + 1) * 128],
                        rhs=qt_sb[0:64, hp, qt * 512:(qt + 1) * 512],
                        start=True, stop=True)
                    nc.tensor.matmul(
                        ps[:, 512:1024],
                        lhsT=kt_sb[64:128, hp, kt * 128:(kt + 1) * 128],
                        rhs=qt_sb[64:128, hp, qt * 512:(qt + 1) * 512],
                        start=True, stop=True, tile_position=(64, 0))
                    es = expp.tile([128, 1024], BF16, tag="es", name="es")
                    nc.scalar.activation(out=es, in_=ps, func=AF.Exp)
                    nc.tensor.matmul(
                        ctxA[0:DK + 1, :], lhsT=v_sb[:, kt, hA, :], rhs=es[:, 0:512],
                        start=(kt == 0), stop=(kt == NK - 1))
                    nc.tensor.matmul(
                        ctxB[0:DK + 1, :], lhsT=v_sb[:, kt, hB, :], rhs=es[:, 512:1024],
                        start=(kt == 0), stop=(kt == NK - 1))
                # softmax denominators for both heads: fast reciprocal, then
                # broadcast across 64 partitions via a DRAM-roundtrip DMA
                # (rows 0-63 <- 1/denomA, rows 64-127 <- 1/denomB)
                recA = stage.tile([1, 512], F32, tag="recA", name="recA")
                recB = stage.tile([1, 512], F32, tag="recB", name="recB")
                nc.vector.reciprocal(recA, ctxA[DK:DK + 1, :])
                nc.vector.reciprocal(recB, ctxB[DK:DK + 1, :])
                rec_dr = drp.tile([2, 512], F32, tag="rec_dr", name="rec_dr")
                nc.gpsimd.dma_start(out=rec_dr[0:1, :], in_=recA)
                nc.gpsimd.dma_start(out=rec_dr[1:2, :], in_=recB)
                bc = stage.tile([128, 512], F32, tag="bc", name="bc")
                nc.gpsimd.dma_start(
                    out=bc,
                    in_=bass.AP(tensor=rec_dr.tensor, offset=rec_dr.offset,
                                ap=[list(rec_dr.ap[0]), [0, DK], [1, 512]]))
                for cps, h in ((ctxA, hA), (ctxB, hB)):
                    r0 = (h % 2) * DK
                    nc.vector.tensor_tensor(
                        out=ctxT_sb[r0:r0 + DK, hp, qt * 512:(qt + 1) * 512],
                        in0=cps[0:DK, :], in1=bc[r0:r0 + DK, :], op=ALU.mult)

                # software-pipelined output projection: emit the PREVIOUS
                # query tile's O-proj here so its (long-latency) normalize
                # dependencies resolved a full head-pair iteration ago and
                # the in-order PE stream never stalls on them
                if qt > 0:
                    oproj(qt - 1, [2 * hp, 2 * hp + 1])
        oproj(NQ - 1, list(range(8)))

    nc.compile()
    return nc


_PROG = None


def _get_prog() -> bass.Bass:
    global _PROG
    if _PROG is None:
        _PROG = build_program()
    return _PROG


def make_in_maps(x, Wq, bq, Wk, bk, Wv, bv, Wo, bo):
    """Build the 8 per-core input dicts from the full (unsharded) inputs."""
    bf = ml_dtypes.bfloat16
    x = np.asarray(x, np.float32)
    scale = np.float32(1.0 / np.sqrt(DK))
    WqT = np.asarray(Wq, np.float32).T * scale   # [d, e], scores scale folded in
    WkT = np.asarray(Wk, np.float32).T
    WvT = np.asarray(Wv, np.float32).T
    WoT = np.asarray(Wo, np.float32).T           # [d_in, e_out]; rows = ctx dims
    bq = np.asarray(bq, np.float32) * scale
    bk = np.asarray(bk, np.float32)
    bv = np.asarray(bv, np.float32)

    xT_b = [np.ascontiguousarray(x[b_].T).astype(bf) for b_ in range(B)]
    in_maps = []
    for c in range(N_CORES):
        b_idx, h2 = divmod(c, TP)
        sl = slice(h2 * EL, (h2 + 1) * EL)
        in_maps.append({
            "xT": xT_b[b_idx],
            "wqT": np.ascontiguousarray(WqT[:, sl]).astype(bf),
            "wkT": np.ascontiguousarray(WkT[:, sl]).astype(bf),
            "wvT": np.ascontiguousarray(WvT[:, sl]).astype(bf),
            "woT": np.ascontiguousarray(WoT[sl, :]).astype(bf),
            "bq": np.ascontiguousarray(bq[sl]).astype(bf),
            "bk": np.ascontiguousarray(bk[sl]).astype(bf),
            "bv": np.ascontiguousarray(bv[sl]).astype(bf),
        })
    return in_maps


def assemble_output(results, bo):
    """Sum TP partials, transpose back to [B, S, D], add output bias."""
    bo32 = np.asarray(bo, np.float32)
    out = np.empty((B, S, D), np.float32)
    for b_idx in range(B):
        acc = results[TP * b_idx]["outT"] + results[TP * b_idx + 1]["outT"]
        out[b_idx] = acc.T + bo32
    return out


def kernel(x, Wq, bq, Wk, bk, Wv, bv, Wo, bo):
    nc = _get_prog()
    in_maps = make_in_maps(x, Wq, bq, Wk, bk, Wv, bv, Wo, bo)
    res = run_bass_kernel_spmd(nc, in_maps, core_ids=list(range(N_CORES)))
    return assemble_output(res.results, bo)


# revision 20
# speedup vs baseline: 1.2875x; 1.2875x over previous
"""Multi-head attention on 8 TRN2 NeuronCores.

Sharding: 4-way data-parallel over batch x 2-way tensor-parallel over heads.
Core c handles batch (c // 2) and heads [8*(c%2), 8*(c%2)+8).

Per-core kernel (feature-major / transposed layouts throughout):
  xT   [1024, 2048]  (bf16, d-major)           -> SBUF
  Q^T, K^T = Wq/Wk slices @ xT + bias           [512, 2048] (e-major, bf16)
             (1/sqrt(dk) folded into Wq, bq on host)
  V    = x @ Wv^T slice + bias, token-major     [2048, 8 heads, 64+1]
         (65th column = ones -> softmax denominator comes free in ctx matmul)
  S^T[k,q] = K^T.T @ Q^T per head               (two heads packed in the
             128-row PE array via tile_position row groups, contraction=64)
  P = exp(S^T)  (scores are small: |S|<~3, so no max-subtraction needed)
  ctx^T[d,q] (+denom row) = V.T @ P             accumulated over 16 k-tiles
  ctx^T normalized by 1/denom (partition_broadcast + DVE multiply)
  outT_partial[e,t] = Wo^T slice.T @ ctx^T      [1024, 2048] f32 -> DRAM

Host: out[b] = (outT_core(2b) + outT_core(2b+1)).T + bo.
"""

import numpy as np
import ml_dtypes
from contextlib import ExitStack

import concourse.bass as bass
import concourse.bacc as bacc
import concourse.mybir as mybir
import concourse.tile as tile
from concourse.bass_utils import run_bass_kernel_spmd

D = 1024          # d_model
HEADS = 16
DK = 64           # head dim
B = 4             # batch
S = 2048          # sequence length
TP = 2            # tensor-parallel ways (over heads)
DP = 4            # data-parallel ways (over batch)
N_CORES = 8
EL = D // TP      # 512 local projection dims
HL = HEADS // TP  # 8 local heads
T = S             # tokens per core (one batch)
KT = D // 128     # 8 contraction tiles for projections
TT = T // 128     # 16 token tiles
NQ = T // 512     # 4 query tiles
NK = T // 128     # 16 key tiles

F32 = mybir.dt.float32
BF16 = mybir.dt.bfloat16
AF = mybir.ActivationFunctionType
ALU = mybir.AluOpType


def _bcast_ap(ap: bass.AP, parts: int) -> bass.AP:
    """Prepend a step-0 partition dim: broadcast a (partition-less or 1-row)
    AP across `parts` partitions for DMA."""
    return bass.AP(tensor=ap.tensor, offset=ap.offset, ap=[[0, parts]] + list(ap.ap))


def build_program() -> bass.Bass:
    nc = bacc.Bacc("TRN2", debug=False)

    xT = nc.dram_tensor("xT", [D, T], BF16, kind="ExternalInput").ap()
    wqT = nc.dram_tensor("wqT", [D, EL], BF16, kind="ExternalInput").ap()
    wkT = nc.dram_tensor("wkT", [D, EL], BF16, kind="ExternalInput").ap()
    wvT = nc.dram_tensor("wvT", [D, EL], BF16, kind="ExternalInput").ap()
    woT = nc.dram_tensor("woT", [EL, D], BF16, kind="ExternalInput").ap()
    bq = nc.dram_tensor("bq", [EL], BF16, kind="ExternalInput").ap()
    bk = nc.dram_tensor("bk", [EL], BF16, kind="ExternalInput").ap()
    bv = nc.dram_tensor("bv", [EL], BF16, kind="ExternalInput").ap()
    outT = nc.dram_tensor("outT", [D, T], F32, kind="ExternalOutput").ap()

    with ExitStack() as ctx:
        tc = ctx.enter_context(tile.TileContext(nc))
        const = ctx.enter_context(tc.tile_pool(name="const", bufs=1))
        xw = ctx.enter_context(tc.tile_pool(name="xw", bufs=1))
        qkv = ctx.enter_context(tc.tile_pool(name="qkv", bufs=1))
        expp = ctx.enter_context(tc.tile_pool(name="expp", bufs=3))
        stage = ctx.enter_context(tc.tile_pool(name="stage", bufs=3))
        psp = ctx.enter_context(tc.tile_pool(name="psp", bufs=2, space="PSUM"))
        ctxp = ctx.enter_context(tc.tile_pool(name="ctxp", bufs=4, space="PSUM"))
        drp = ctx.enter_context(tc.tile_pool(name="drp", bufs=3, space="DRAM"))

        # ---------------- loads ----------------
        xt_sb = xw.tile([128, KT, T], BF16)          # [p, kt, t]
        for kt in range(KT):
            nc.sync.dma_start(out=xt_sb[:, kt, :], in_=xT[kt * 128:(kt + 1) * 128, :])
        wq_sb = xw.tile([128, KT, EL], BF16)
        wk_sb = xw.tile([128, KT, EL], BF16)
        wv_sb = xw.tile([128, KT, EL], BF16)
        for w_sb, w_dram in ((wq_sb, wqT), (wk_sb, wkT), (wv_sb, wvT)):
            for kt in range(KT):
                nc.sync.dma_start(out=w_sb[:, kt, :], in_=w_dram[kt * 128:(kt + 1) * 128, :])
        wo_sb = xw.tile([128, EL // 128, D], BF16)
        for kt in range(EL // 128):
            nc.sync.dma_start(out=wo_sb[:, kt, :], in_=woT[kt * 128:(kt + 1) * 128, :])

        # biases live on one partition row; folded into the matmuls as a
        # K=1 rank-1 update (lhsT/rhs of ones), avoiding extra DVE ops
        bq_sb = const.tile([1, EL], BF16)
        nc.sync.dma_start(out=bq_sb, in_=_bcast_ap(bq, 1))
        bk_sb = const.tile([1, EL], BF16)
        nc.sync.dma_start(out=bk_sb, in_=_bcast_ap(bk, 1))
        bv_sb = const.tile([1, EL], BF16)
        nc.sync.dma_start(out=bv_sb, in_=_bcast_ap(bv, 1))
        ones_sb = const.tile([1, 512], BF16)
        nc.vector.memset(ones_sb, 1.0)

        # ---------------- Q/K/V projections ----------------
        # Order: Q[hp0], K[hp0] first so attention on head-pair 0 (and its
        # exp stream on ScalarE, the bottleneck engine) starts ~15us in,
        # then V (needed by the first ctx matmuls), then remaining Q/K.
        qt_sb = qkv.tile([128, EL // 128, T], BF16)   # [p(=e within hp), hp, t]
        kt_sb = qkv.tile([128, EL // 128, T], BF16)
        v_sb = qkv.tile([128, TT, HL, DK + 1], BF16)  # [p(=t within tt), tt, h, dk+ones]
        nc.vector.memset(v_sb[:, :, :, DK:DK + 1], 1.0)

        def qk_proj(hp):
            for w_sb, b_sb, dst in ((wq_sb, bq_sb, qt_sb), (wk_sb, bk_sb, kt_sb)):
                for nt2 in range(T // 1024):
                    ps = psp.tile([128, 1024], F32, tag="ps", name="ps")
                    for half in range(2):
                        t0 = (nt2 * 2 + half) * 512
                        # bias via rank-1 update: ps = b[e] * ones[t]
                        nc.tensor.matmul(
                            ps[:, half * 512:(half + 1) * 512],
                            lhsT=b_sb[0:1, hp * 128:(hp + 1) * 128],
                            rhs=ones_sb[0:1, :],
                            start=True, stop=False)
                        for kt in range(KT):
                            nc.tensor.matmul(
                                ps[:, half * 512:(half + 1) * 512],
                                lhsT=w_sb[:, kt, hp * 128:(hp + 1) * 128],
                                rhs=xt_sb[:, kt, t0:t0 + 512],
                                start=False, stop=(kt == KT - 1))
                    nc.vector.tensor_copy(
                        out=dst[:, hp, nt2 * 1024:(nt2 + 1) * 1024],
                        in_=ps)

        def v_proj():
            for tt in range(TT):
                psv = psp.tile([128, 512], F32, tag="ps", name="psv")
                # bias via rank-1 update: psv = ones[t] * bv[e]
                nc.tensor.matmul(
                    psv, lhsT=ones_sb[0:1, 0:128], rhs=bv_sb[0:1, :],
                    start=True, stop=False)
                for kt in range(KT):
                    nc.tensor.matmul(
                        psv,
                        lhsT=xt_sb[:, kt, tt * 128:(tt + 1) * 128],
                        rhs=wv_sb[:, kt, :],
                        start=False, stop=(kt == KT - 1))
                nc.vector.tensor_copy(
                    out=v_sb[:, tt, :, 0:DK],
                    in_=psv.rearrange("p (h d) -> p h d", h=HL))

        qk_proj(0)
        v_proj()
        for hp in range(1, EL // 128):
            qk_proj(hp)

        # ---------------- attention + output projection ----------------
        ctxT_sb = qkv.tile([128, EL // 128, T], BF16)  # [p(=d within hp), hp, q]
        outT_r = outT.rearrange("(E p) t -> p E t", p=128)  # [128, 8, 2048]

        def oproj(qt, ets):
            # output projection, one [128, 512] out tile per e-tile
            for et in ets:
                pso = ctxp.tile([128, 512], F32, tag="ctx", name="pso")
                for hp in range(EL // 128):
                    nc.tensor.matmul(
                        pso,
                        lhsT=wo_sb[:, hp, et * 128:(et + 1) * 128],
                        rhs=ctxT_sb[:, hp, qt * 512:(qt + 1) * 512],
                        start=(hp == 0), stop=(hp == EL // 128 - 1))
                ot = stage.tile([128, 512], F32, tag="ot", name="ot")
                nc.vector.tensor_copy(ot, pso)
                nc.sync.dma_start(
                    out=outT_r[:, et, qt * 512:(qt + 1) * 512],
                    in_=ot)

        for qt in range(NQ):
            for hp in range(EL // 128):
                hA, hB = 2 * hp, 2 * hp + 1
                ctxA = ctxp.tile([128, 512], F32, tag="ctx", name="ctxA")
                ctxB = ctxp.tile([128, 512], F32, tag="ctx", name="ctxB")
                for kt in range(NK):
                    ps = psp.tile([128, 1024], F32, tag="ps", name="pss")
                    # S^T tiles for two heads packed into row-groups 0-63 / 64-127
                    nc.tensor.matmul(
                        ps[:, 0:512],
                        lhsT=kt_sb[0:64, hp, kt * 128:(kt + 1) * 128],
                        rhs=qt_sb[0:64, hp, qt * 512:(qt + 1) * 512],
                        start=True, stop=True)
                    nc.tensor.matmul(
                        ps[:, 512:1024],
                        lhsT=kt_sb[64:128, hp, kt * 128:(kt + 1) * 128],
                        rhs=qt_sb[64:128, hp, qt * 512:(qt + 1) * 512],
                        start=True, stop=True, tile_position=(64, 0))
                    es = expp.tile([128, 1024], BF16, tag="es", name="es")
                    nc.scalar.activation(out=es, in_=ps, func=AF.Exp)
                    nc.tensor.matmul(
                        ctxA[0:DK + 1, :], lhsT=v_sb[:, kt, hA, :], rhs=es[:, 0:512],
                        start=(kt == 0), stop=(kt == NK - 1))
                    nc.tensor.matmul(
                        ctxB[0:DK + 1, :], lhsT=v_sb[:, kt, hB, :], rhs=es[:, 512:1024],
                        start=(kt == 0), stop=(kt == NK - 1))
                # Copy ctx+denom out of PSUM immediately: this is the only
                # reader of the psum banks, so the (in-order) PE stream's next
                # matmuls get their banks back after one fast DVE copy instead
                # of waiting for the whole reciprocal/broadcast chain.
                cA = stage.tile([DK + 1, 512], F32, tag="cA", name="cA")
                cB = stage.tile([DK + 1, 512], F32, tag="cB", name="cB")
                nc.vector.tensor_copy(cA, ctxA[0:DK + 1, :])
                nc.vector.tensor_copy(cB, ctxB[0:DK + 1, :])
                # softmax denominators for both heads: reciprocal, then
                # broadcast across 64 partitions via a DRAM-roundtrip DMA
                # (rows 0-63 <- 1/denomA, rows 64-127 <- 1/denomB)
                recA = stage.tile([1, 512], F32, tag="recA", name="recA")
                recB = stage.tile([1, 512], F32, tag="recB", name="recB")
                nc.vector.reciprocal(recA, cA[DK:DK + 1, :])
                nc.vector.reciprocal(recB, cB[DK:DK + 1, :])
                rec_dr = drp.tile([2, 512], F32, tag="rec_dr", name="rec_dr")
                nc.gpsimd.dma_start(out=rec_dr[0:1, :], in_=recA)
                nc.gpsimd.dma_start(out=rec_dr[1:2, :], in_=recB)
                bcA = stage.tile([DK, 512], F32, tag="bcA", name="bcA")
                bcB = stage.tile([DK, 512], F32, tag="bcB", name="bcB")
                nc.gpsimd.dma_start(out=bcA, in_=_bcast_ap(rec_dr[0, :], DK))
                nc.gpsimd.dma_start(out=bcB, in_=_bcast_ap(rec_dr[1, :], DK))
                for csb, bc, h in ((cA, bcA, hA), (cB, bcB, hB)):
                    r0 = (h % 2) * DK
                    nc.vector.tensor_tensor(
                        out=ctxT_sb[r0:r0 + DK, hp, qt * 512:(qt + 1) * 512],
                        in0=csb[0:DK, :], in1=bc, op=ALU.mult)

                # software-pipelined output projection: emit the PREVIOUS
                # query tile's O-proj here so its (long-latency) normalize
                # dependencies resolved a full head-pair iteration ago and
                # the in-order PE stream never stalls on them
                if qt > 0:
                    oproj(qt - 1, [2 * hp, 2 * hp + 1])
        oproj(NQ - 1, list(range(8)))

    nc.compile()
    return nc


_PROG = None


def _get_prog() -> bass.Bass:
    global _PROG
    if _PROG is None:
        _PROG = build_program()
    return _PROG


def make_in_maps(x, Wq, bq, Wk, bk, Wv, bv, Wo, bo):
    """Build the 8 per-core input dicts from the full (unsharded) inputs."""
    bf = ml_dtypes.bfloat16
    x = np.asarray(x, np.float32)
    scale = np.float32(1.0 / np.sqrt(DK))
    WqT = np.asarray(Wq, np.float32).T * scale   # [d, e], scores scale folded in
    WkT = np.asarray(Wk, np.float32).T
    WvT = np.asarray(Wv, np.float32).T
    WoT = np.asarray(Wo, np.float32).T           # [d_in, e_out]; rows = ctx dims
    bq = np.asarray(bq, np.float32) * scale
    bk = np.asarray(bk, np.float32)
    bv = np.asarray(bv, np.float32)

    xT_b = [np.ascontiguousarray(x[b_].T).astype(bf) for b_ in range(B)]
    in_maps = []
    for c in range(N_CORES):
        b_idx, h2 = divmod(c, TP)
        sl = slice(h2 * EL, (h2 + 1) * EL)
        in_maps.append({
            "xT": xT_b[b_idx],
            "wqT": np.ascontiguousarray(WqT[:, sl]).astype(bf),
            "wkT": np.ascontiguousarray(WkT[:, sl]).astype(bf),
            "wvT": np.ascontiguousarray(WvT[:, sl]).astype(bf),
            "woT": np.ascontiguousarray(WoT[sl, :]).astype(bf),
            "bq": np.ascontiguousarray(bq[sl]).astype(bf),
            "bk": np.ascontiguousarray(bk[sl]).astype(bf),
            "bv": np.ascontiguousarray(bv[sl]).astype(bf),
        })
    return in_maps


def assemble_output(results, bo):
    """Sum TP partials, transpose back to [B, S, D], add output bias."""
    bo32 = np.asarray(bo, np.float32)
    out = np.empty((B, S, D), np.float32)
    for b_idx in range(B):
        acc = results[TP * b_idx]["outT"] + results[TP * b_idx + 1]["outT"]
        out[b_idx] = acc.T + bo32
    return out


def kernel(x, Wq, bq, Wk, bk, Wv, bv, Wo, bo):
    nc = _get_prog()
    in_maps = make_in_maps(x, Wq, bq, Wk, bk, Wv, bv, Wo, bo)
    res = run_bass_kernel_spmd(nc, in_maps, core_ids=list(range(N_CORES)))
    return assemble_output(res.results, bo)


# revision 21
# speedup vs baseline: 1.3121x; 1.0191x over previous
"""Multi-head attention on 8 TRN2 NeuronCores.

Sharding: 4-way data-parallel over batch x 2-way tensor-parallel over heads.
Core c handles batch (c // 2) and heads [8*(c%2), 8*(c%2)+8).

Per-core kernel (feature-major / transposed layouts throughout):
  xT   [1024, 2048]  (bf16, d-major)           -> SBUF
  Q^T, K^T = Wq/Wk slices @ xT + bias           [512, 2048] (e-major, bf16)
             (1/sqrt(dk) folded into Wq, bq on host)
  V    = x @ Wv^T slice + bias, token-major     [2048, 8 heads, 64+1]
         (65th column = ones -> softmax denominator comes free in ctx matmul)
  S^T[k,q] = K^T.T @ Q^T per head               (two heads packed in the
             128-row PE array via tile_position row groups, contraction=64)
  P = exp(S^T)  (scores are small: |S|<~3, so no max-subtraction needed)
  ctx^T[d,q] (+denom row) = V.T @ P             accumulated over 16 k-tiles
  ctx^T normalized by 1/denom (DRAM-roundtrip broadcast + DVE multiply)
  outT_partial[e,t] = Wo^T slice.T @ ctx^T      [1024, 2048] f32 -> DRAM

Host: out[b] = (outT_core(2b) + outT_core(2b+1)).T + bo.

Scheduling notes (engines execute their instruction streams IN ORDER):
  - ScalarE (exp, 128 lanes @ 1.2 GHz, ~278us of work) is the bottleneck;
    everything is arranged to keep its stream fed from ~20us onward.
  - Projections for head-pairs 1-3 are emitted INSIDE the first query tile's
    attention loop so the PE fills its slack without delaying the exp stream.
  - The O-projection runs one query tile behind, so its dependency on the
    (long-latency) softmax-normalize chain is always pre-resolved.
  - ctx PSUM banks are freed by a single fast DVE copy; the reciprocal /
    broadcast / multiply chain runs SBUF-side off the PE critical path.
"""

import numpy as np
import ml_dtypes
from contextlib import ExitStack

import concourse.bass as bass
import concourse.bacc as bacc
import concourse.mybir as mybir
import concourse.tile as tile
from concourse.bass_utils import run_bass_kernel_spmd

D = 1024          # d_model
HEADS = 16
DK = 64           # head dim
B = 4             # batch
S = 2048          # sequence length
TP = 2            # tensor-parallel ways (over heads)
DP = 4            # data-parallel ways (over batch)
N_CORES = 8
EL = D // TP      # 512 local projection dims
HL = HEADS // TP  # 8 local heads
HP = EL // 128    # 4 head-pairs per core
T = S             # tokens per core (one batch)
KT = D // 128     # 8 contraction tiles for projections
TT = T // 128     # 16 token tiles
NQ = T // 512     # 4 query tiles
NK = T // 128     # 16 key tiles

F32 = mybir.dt.float32
BF16 = mybir.dt.bfloat16
AF = mybir.ActivationFunctionType
ALU = mybir.AluOpType


def _bcast_ap(ap: bass.AP, parts: int) -> bass.AP:
    """Prepend a step-0 partition dim: broadcast a (partition-less or 1-row)
    AP across `parts` partitions for DMA. DRAM-side only."""
    return bass.AP(tensor=ap.tensor, offset=ap.offset, ap=[[0, parts]] + list(ap.ap))


def build_program() -> bass.Bass:
    nc = bacc.Bacc("TRN2", debug=False)

    xT = nc.dram_tensor("xT", [D, T], BF16, kind="ExternalInput").ap()
    wqT = nc.dram_tensor("wqT", [D, EL], BF16, kind="ExternalInput").ap()
    wkT = nc.dram_tensor("wkT", [D, EL], BF16, kind="ExternalInput").ap()
    wvT = nc.dram_tensor("wvT", [D, EL], BF16, kind="ExternalInput").ap()
    woT = nc.dram_tensor("woT", [EL, D], BF16, kind="ExternalInput").ap()
    bq = nc.dram_tensor("bq", [EL], F32, kind="ExternalInput").ap()
    bk = nc.dram_tensor("bk", [EL], F32, kind="ExternalInput").ap()
    bv = nc.dram_tensor("bv", [EL], F32, kind="ExternalInput").ap()
    outT = nc.dram_tensor("outT", [D, T], F32, kind="ExternalOutput").ap()

    with ExitStack() as ctx:
        tc = ctx.enter_context(tile.TileContext(nc))
        const = ctx.enter_context(tc.tile_pool(name="const", bufs=1))
        xw = ctx.enter_context(tc.tile_pool(name="xw", bufs=1))
        qkv = ctx.enter_context(tc.tile_pool(name="qkv", bufs=1))
        expp = ctx.enter_context(tc.tile_pool(name="expp", bufs=3))
        stage = ctx.enter_context(tc.tile_pool(name="stage", bufs=3))
        psp = ctx.enter_context(tc.tile_pool(name="psp", bufs=2, space="PSUM"))
        ctxp = ctx.enter_context(tc.tile_pool(name="ctxp", bufs=4, space="PSUM"))
        drp = ctx.enter_context(tc.tile_pool(name="drp", bufs=3, space="DRAM"))

        # ---------------- loads (most-urgent first) ----------------
        xt_sb = xw.tile([128, KT, T], BF16)          # [p, kt, t]
        for kt in range(KT):
            nc.sync.dma_start(out=xt_sb[:, kt, :], in_=xT[kt * 128:(kt + 1) * 128, :])
        wq_sb = xw.tile([128, KT, EL], BF16)
        wk_sb = xw.tile([128, KT, EL], BF16)
        wv_sb = xw.tile([128, KT, EL], BF16)
        for w_sb, w_dram in ((wq_sb, wqT), (wk_sb, wkT), (wv_sb, wvT)):
            for kt in range(KT):
                nc.sync.dma_start(out=w_sb[:, kt, :], in_=w_dram[kt * 128:(kt + 1) * 128, :])
        bq_sb = const.tile([128, HP], F32)
        nc.sync.dma_start(out=bq_sb, in_=bq.rearrange("(a p) -> p a", p=128))
        bk_sb = const.tile([128, HP], F32)
        nc.sync.dma_start(out=bk_sb, in_=bk.rearrange("(a p) -> p a", p=128))
        bv_sb = const.tile([128, HL, DK], F32)
        nc.sync.dma_start(out=bv_sb, in_=_bcast_ap(bv.rearrange("(h d) -> h d", h=HL), 128))
        # Wo is needed only by the first O-proj, a good ~100us in
        wo_sb = xw.tile([128, HP, D], BF16)
        for kt in range(HP):
            nc.sync.dma_start(out=wo_sb[:, kt, :], in_=woT[kt * 128:(kt + 1) * 128, :])

        # ---------------- projection + attention bodies ----------------
        qt_sb = qkv.tile([128, HP, T], BF16)   # [p(=e within hp), hp, t]
        kt_sb = qkv.tile([128, HP, T], BF16)
        v_sb = qkv.tile([128, TT, HL, DK + 1], BF16)  # [p(=t in tt), tt, h, dk+ones]
        nc.vector.memset(v_sb[:, :, :, DK:DK + 1], 1.0)
        ctxT_sb = qkv.tile([128, HP, T], BF16)  # [p(=d within hp), hp, q]
        outT_r = outT.rearrange("(E p) t -> p E t", p=128)  # [128, 8, 2048]

        def qk_proj(hp):
            for w_sb, b_sb, dst in ((wq_sb, bq_sb, qt_sb), (wk_sb, bk_sb, kt_sb)):
                for nt2 in range(T // 1024):
                    ps = psp.tile([128, 1024], F32, tag="ps", name="ps")
                    for half in range(2):
                        t0 = (nt2 * 2 + half) * 512
                        for kt in range(KT):
                            nc.tensor.matmul(
                                ps[:, half * 512:(half + 1) * 512],
                                lhsT=w_sb[:, kt, hp * 128:(hp + 1) * 128],
                                rhs=xt_sb[:, kt, t0:t0 + 512],
                                start=(kt == 0), stop=(kt == KT - 1))
                    nc.vector.tensor_scalar_add(
                        out=dst[:, hp, nt2 * 1024:(nt2 + 1) * 1024],
                        in0=ps, scalar1=b_sb[:, hp:hp + 1])

        def v_proj():
            for tt in range(TT):
                psv = psp.tile([128, 512], F32, tag="ps", name="psv")
                for kt in range(KT):
                    nc.tensor.matmul(
                        psv,
                        lhsT=xt_sb[:, kt, tt * 128:(tt + 1) * 128],
                        rhs=wv_sb[:, kt, :],
                        start=(kt == 0), stop=(kt == KT - 1))
                nc.vector.tensor_tensor(
                    out=v_sb[:, tt, :, 0:DK],
                    in0=psv.rearrange("p (h d) -> p h d", h=HL),
                    in1=bv_sb, op=ALU.add)

        def oproj(qt, ets):
            # output projection, one [128, 512] out tile per e-tile
            for et in ets:
                pso = ctxp.tile([128, 512], F32, tag="ctx", name="pso")
                for hp in range(HP):
                    nc.tensor.matmul(
                        pso,
                        lhsT=wo_sb[:, hp, et * 128:(et + 1) * 128],
                        rhs=ctxT_sb[:, hp, qt * 512:(qt + 1) * 512],
                        start=(hp == 0), stop=(hp == HP - 1))
                ot = stage.tile([128, 512], F32, tag="ot", name="ot")
                nc.vector.tensor_copy(ot, pso)
                nc.sync.dma_start(
                    out=outT_r[:, et, qt * 512:(qt + 1) * 512],
                    in_=ot)

        # ---------------- fused schedule ----------------
        qk_proj(0)
        v_proj()

        for qt in range(NQ):
            for hp in range(HP):
                hA, hB = 2 * hp, 2 * hp + 1
                ctxA = ctxp.tile([128, 512], F32, tag="ctx", name="ctxA")
                ctxB = ctxp.tile([128, 512], F32, tag="ctx", name="ctxB")
                for kt in range(NK):
                    ps = psp.tile([128, 1024], F32, tag="ps", name="pss")
                    # S^T tiles for two heads packed into row-groups 0-63 / 64-127
                    nc.tensor.matmul(
                        ps[:, 0:512],
                        lhsT=kt_sb[0:64, hp, kt * 128:(kt + 1) * 128],
                        rhs=qt_sb[0:64, hp, qt * 512:(qt + 1) * 512],
                        start=True, stop=True)
                    nc.tensor.matmul(
                        ps[:, 512:1024],
                        lhsT=kt_sb[64:128, hp, kt * 128:(kt + 1) * 128],
                        rhs=qt_sb[64:128, hp, qt * 512:(qt + 1) * 512],
                        start=True, stop=True, tile_position=(64, 0))
                    es = expp.tile([128, 1024], BF16, tag="es", name="es")
                    nc.scalar.activation(out=es, in_=ps, func=AF.Exp)
                    nc.tensor.matmul(
                        ctxA[0:DK + 1, :], lhsT=v_sb[:, kt, hA, :], rhs=es[:, 0:512],
                        start=(kt == 0), stop=(kt == NK - 1))
                    nc.tensor.matmul(
                        ctxB[0:DK + 1, :], lhsT=v_sb[:, kt, hB, :], rhs=es[:, 512:1024],
                        start=(kt == 0), stop=(kt == NK - 1))

                # Free the ctx psum banks with one fast DVE copy each; the
                # reciprocal/broadcast/multiply chain then runs SBUF-side.
                cA = stage.tile([DK + 1, 512], F32, tag="cA", name="cA")
                cB = stage.tile([DK + 1, 512], F32, tag="cB", name="cB")
                nc.vector.tensor_copy(cA, ctxA[0:DK + 1, :])
                nc.vector.tensor_copy(cB, ctxB[0:DK + 1, :])

                # Pipelined PE filler work whose dependencies are long
                # resolved: projections for the next head-pair during the
                # first query tile, the previous tile's O-proj afterwards.
                if qt == 0:
                    if hp + 1 < HP:
                        qk_proj(hp + 1)
                else:
                    oproj(qt - 1, [2 * hp, 2 * hp + 1])

                # softmax denominators for both heads: reciprocal, then
                # broadcast across 64 partitions via a DRAM-roundtrip DMA
                recA = stage.tile([1, 512], F32, tag="recA", name="recA")
                recB = stage.tile([1, 512], F32, tag="recB", name="recB")
                nc.vector.reciprocal(recA, cA[DK:DK + 1, :])
                nc.vector.reciprocal(recB, cB[DK:DK + 1, :])
                rec_dr = drp.tile([2, 512], F32, tag="rec_dr", name="rec_dr")
                nc.gpsimd.dma_start(out=rec_dr[0:1, :], in_=recA)
                nc.gpsimd.dma_start(out=rec_dr[1:2, :], in_=recB)
                bcA = stage.tile([DK, 512], F32, tag="bcA", name="bcA")
                bcB = stage.tile([DK, 512], F32, tag="bcB", name="bcB")
                nc.gpsimd.dma_start(out=bcA, in_=_bcast_ap(rec_dr[0, :], DK))
                nc.gpsimd.dma_start(out=bcB, in_=_bcast_ap(rec_dr[1, :], DK))
                for csb, bc, h in ((cA, bcA, hA), (cB, bcB, hB)):
                    r0 = (h % 2) * DK
                    nc.vector.tensor_tensor(
                        out=ctxT_sb[r0:r0 + DK, hp, qt * 512:(qt + 1) * 512],
                        in0=csb[0:DK, :], in1=bc, op=ALU.mult)

        oproj(NQ - 1, list(range(8)))

    nc.compile()
    return nc


_PROG = None


def _get_prog() -> bass.Bass:
    global _PROG
    if _PROG is None:
        _PROG = build_program()
    return _PROG


def make_in_maps(x, Wq, bq, Wk, bk, Wv, bv, Wo, bo):
    """Build the 8 per-core input dicts from the full (unsharded) inputs."""
    bf = ml_dtypes.bfloat16
    x = np.asarray(x, np.float32)
    scale = np.float32(1.0 / np.sqrt(DK))
    WqT = np.asarray(Wq, np.float32).T * scale   # [d, e], scores scale folded in
    WkT = np.asarray(Wk, np.float32).T
    WvT = np.asarray(Wv, np.float32).T
    WoT = np.asarray(Wo, np.float32).T           # [d_in, e_out]; rows = ctx dims
    bq = np.asarray(bq, np.float32) * scale
    bk = np.asarray(bk, np.float32)
    bv = np.asarray(bv, np.float32)

    xT_b = [np.ascontiguousarray(x[b_].T).astype(bf) for b_ in range(B)]
    in_maps = []
    for c in range(N_CORES):
        b_idx, h2 = divmod(c, TP)
        sl = slice(h2 * EL, (h2 + 1) * EL)
        in_maps.append({
            "xT": xT_b[b_idx],
            "wqT": np.ascontiguousarray(WqT[:, sl]).astype(bf),
            "wkT": np.ascontiguousarray(WkT[:, sl]).astype(bf),
            "wvT": np.ascontiguousarray(WvT[:, sl]).astype(bf),
            "woT": np.ascontiguousarray(WoT[sl, :]).astype(bf),
            "bq": np.ascontiguousarray(bq[sl]),
            "bk": np.ascontiguousarray(bk[sl]),
            "bv": np.ascontiguousarray(bv[sl]),
        })
    return in_maps


def assemble_output(results, bo):
    """Sum TP partials, transpose back to [B, S, D], add output bias."""
    bo32 = np.asarray(bo, np.float32)
    out = np.empty((B, S, D), np.float32)
    for b_idx in range(B):
        acc = results[TP * b_idx]["outT"] + results[TP * b_idx + 1]["outT"]
        out[b_idx] = acc.T + bo32
    return out


def kernel(x, Wq, bq, Wk, bk, Wv, bv, Wo, bo):
    nc = _get_prog()
    in_maps = make_in_maps(x, Wq, bq, Wk, bk, Wv, bv, Wo, bo)
    res = run_bass_kernel_spmd(nc, in_maps, core_ids=list(range(N_CORES)))
    return assemble_output(res.results, bo)


# revision 31
# speedup vs baseline: 1.4375x; 1.0956x over previous
"""Multi-head attention on 8 TRN2 NeuronCores.

Sharding: 4-way data-parallel over batch x 2-way tensor-parallel over heads.
Core c handles batch (c // 2) and heads [8*(c%2), 8*(c%2)+8).

Per-core kernel (feature-major / transposed layouts throughout):
  xT   [1024, 2048]  (bf16, d-major)           -> SBUF
  Q^T, K^T = Wq/Wk slices @ xT + bias           [512, 2048] (e-major, bf16)
             (1/sqrt(dk) folded into Wq, bq on host)
  V    = x @ Wv^T slice + bias, token-major     [2048, 8 heads, 64+1]
         (65th column = ones -> softmax denominator comes free in ctx matmul)
  S^T[k,q] = K^T.T @ Q^T per head               (two heads packed in the
             128-row PE array via tile_position row groups, contraction=64)
  P = exp(S^T)  (scores are small: |S|<~3, so no max-subtraction needed)
  ctx^T[d,q] (+denom row) = V.T @ P             accumulated over 16 k-tiles
  ctx^T normalized by 1/denom (DRAM-roundtrip broadcast + DVE multiply)
  outT_partial[e,t] = Wo^T slice.T @ ctx^T      [1024, 2048] f32 -> DRAM

Host: out[b] = (outT_core(2b) + outT_core(2b+1)).T + bo.

Scheduling notes (engines execute their instruction streams IN ORDER):
  - ScalarE (exp, 128 lanes @ 1.2 GHz, ~278us of work) is the bottleneck;
    everything is arranged to keep its stream fed from ~20us onward.
  - Projections for head-pairs 1-3 are emitted INSIDE the first query tile's
    attention loop so the PE fills its slack without delaying the exp stream.
  - The O-projection runs one query tile behind, so its dependency on the
    (long-latency) softmax-normalize chain is always pre-resolved.
  - ctx PSUM banks are freed by a single fast DVE copy; the reciprocal /
    broadcast / multiply chain runs SBUF-side off the PE critical path.
"""

import numpy as np
import ml_dtypes
from contextlib import ExitStack

import concourse.bass as bass
import concourse.bacc as bacc
import concourse.bass_utils as _bu
import concourse.mybir as mybir
import concourse.tile as tile
from concourse.bass_utils import run_bass_kernel_spmd




D = 1024          # d_model
HEADS = 16
DK = 64           # head dim
B = 4             # batch
S = 2048          # sequence length
TP = 2            # tensor-parallel ways (over heads)
DP = 4            # data-parallel ways (over batch)
N_CORES = 8
EL = D // TP      # 512 local projection dims
HL = HEADS // TP  # 8 local heads
HP = EL // 128    # 4 head-pairs per core
T = S             # tokens per core (one batch)
KT = D // 128     # 8 contraction tiles for projections
TT = T // 128     # 16 token tiles
NQ = T // 512     # 4 query tiles
NK = T // 128     # 16 key tiles

F32 = mybir.dt.float32
BF16 = mybir.dt.bfloat16
AF = mybir.ActivationFunctionType
ALU = mybir.AluOpType


def _bcast_ap(ap: bass.AP, parts: int) -> bass.AP:
    """Prepend a step-0 partition dim: broadcast a (partition-less or 1-row)
    AP across `parts` partitions for DMA. DRAM-side only."""
    return bass.AP(tensor=ap.tensor, offset=ap.offset, ap=[[0, parts]] + list(ap.ap))


def build_program() -> bass.Bass:
    nc = bacc.Bacc("TRN2", debug=False)

    xT = nc.dram_tensor("xT", [D, T], BF16, kind="ExternalInput").ap()
    wqT = nc.dram_tensor("wqT", [D, EL], BF16, kind="ExternalInput").ap()
    wkT = nc.dram_tensor("wkT", [D, EL], BF16, kind="ExternalInput").ap()
    wvT = nc.dram_tensor("wvT", [D, EL], BF16, kind="ExternalInput").ap()
    woT = nc.dram_tensor("woT", [EL, D], BF16, kind="ExternalInput").ap()
    bq = nc.dram_tensor("bq", [EL], F32, kind="ExternalInput").ap()
    bk = nc.dram_tensor("bk", [EL], F32, kind="ExternalInput").ap()
    bv = nc.dram_tensor("bv", [EL], F32, kind="ExternalInput").ap()
    outT = nc.dram_tensor("outT", [D, T], F32, kind="ExternalOutput").ap()

    with ExitStack() as ctx:
        tc = ctx.enter_context(tile.TileContext(nc))
        const = ctx.enter_context(tc.tile_pool(name="const", bufs=1))
        xw = ctx.enter_context(tc.tile_pool(name="xw", bufs=1))
        qkv = ctx.enter_context(tc.tile_pool(name="qkv", bufs=1))
        expp = ctx.enter_context(tc.tile_pool(name="expp", bufs=4))
        stage = ctx.enter_context(tc.tile_pool(name="stage", bufs=3))
        psp = ctx.enter_context(tc.tile_pool(name="psp", bufs=2, space="PSUM"))
        ctxp = ctx.enter_context(tc.tile_pool(name="ctxp", bufs=2, space="PSUM"))
        fillp = ctx.enter_context(tc.tile_pool(name="fillp", bufs=2, space="PSUM"))
        drp = ctx.enter_context(tc.tile_pool(name="drp", bufs=3, space="DRAM"))

        # ---------------- loads (most-urgent first) ----------------
        # interleave x/Wq/Wk per k-tile so the first projection matmuls can
        # start while the rest of the inputs stream in
        xt_sb = xw.tile([128, KT, T], BF16)          # [p, kt, t]
        wq_sb = xw.tile([128, KT, EL], BF16)
        wk_sb = xw.tile([128, KT, EL], BF16)
        wv_sb = xw.tile([128, KT, EL], BF16)
        for kt in range(KT):
            nc.sync.dma_start(out=xt_sb[:, kt, :], in_=xT[kt * 128:(kt + 1) * 128, :])
            nc.sync.dma_start(out=wq_sb[:, kt, :], in_=wqT[kt * 128:(kt + 1) * 128, :])
            nc.sync.dma_start(out=wk_sb[:, kt, :], in_=wkT[kt * 128:(kt + 1) * 128, :])
        for kt in range(KT):
            nc.sync.dma_start(out=wv_sb[:, kt, :], in_=wvT[kt * 128:(kt + 1) * 128, :])
        bq_sb = const.tile([128, HP], F32)
        nc.sync.dma_start(out=bq_sb, in_=bq.rearrange("(a p) -> p a", p=128))
        bk_sb = const.tile([128, HP], F32)
        nc.sync.dma_start(out=bk_sb, in_=bk.rearrange("(a p) -> p a", p=128))
        bv_sb = const.tile([128, HL, DK], F32)
        nc.sync.dma_start(out=bv_sb, in_=_bcast_ap(bv.rearrange("(h d) -> h d", h=HL), 128))
        # Wo is needed only by the first O-proj, a good ~100us in
        wo_sb = xw.tile([128, HP, D], BF16)
        for kt in range(HP):
            nc.sync.dma_start(out=wo_sb[:, kt, :], in_=woT[kt * 128:(kt + 1) * 128, :])

        # ---------------- projection + attention bodies ----------------
        qt_sb = qkv.tile([128, HP, T], BF16)   # [p(=e within hp), hp, t]
        kt_sb = qkv.tile([128, HP, T], BF16)
        v_sb = qkv.tile([128, TT, HL, DK + 1], BF16)  # [p(=t in tt), tt, h, dk+ones]
        nc.vector.memset(v_sb[:, :, :, DK:DK + 1], 1.0)
        ctxT_sb = qkv.tile([128, HP, T], BF16)  # [p(=d within hp), hp, q]
        outT_r = outT.rearrange("(E p) t -> p E t", p=128)  # [128, 8, 2048]

        # --- filler units: small PE work packets fed into attention slack ---
        def qk_unit(hp, which, half):
            # one [128, 512] tile of the Q or K projection for head-pair hp
            w_sb, b_sb, dst = ((wq_sb, bq_sb, qt_sb) if which == 0
                               else (wk_sb, bk_sb, kt_sb))
            t0 = half * 512
            fp = fillp.tile([128, 512], F32, tag="fill", name="fp")
            for kt in range(KT):
                nc.tensor.matmul(
                    fp,
                    lhsT=w_sb[:, kt, hp * 128:(hp + 1) * 128],
                    rhs=xt_sb[:, kt, t0:t0 + 512],
                    start=(kt == 0), stop=(kt == KT - 1))
            nc.vector.tensor_scalar_add(
                out=dst[:, hp, t0:t0 + 512], in0=fp, scalar1=b_sb[:, hp:hp + 1])

        def v_unit(tt):
            # one token-tile of the V projection (all 8 heads)
            fp = fillp.tile([128, 512], F32, tag="fill", name="fpv")
            for kt in range(KT):
                nc.tensor.matmul(
                    fp,
                    lhsT=xt_sb[:, kt, tt * 128:(tt + 1) * 128],
                    rhs=wv_sb[:, kt, :],
                    start=(kt == 0), stop=(kt == KT - 1))
            nc.vector.tensor_tensor(
                out=v_sb[:, tt, :, 0:DK],
                in0=fp.rearrange("p (h d) -> p h d", h=HL),
                in1=bv_sb, op=ALU.add)

        def oproj_unit(qt, et):
            # one e-tile of the output projection for query tile qt
            fp = fillp.tile([128, 512], F32, tag="fill", name="fpo")
            for hp in range(HP):
                nc.tensor.matmul(
                    fp,
                    lhsT=wo_sb[:, hp, et * 128:(et + 1) * 128],
                    rhs=ctxT_sb[:, hp, qt * 512:(qt + 1) * 512],
                    start=(hp == 0), stop=(hp == HP - 1))
            ot = stage.tile([128, 512], F32, tag="ot", name="ot")
            nc.vector.tensor_copy(ot, fp)
            nc.sync.dma_start(out=outT_r[:, et, qt * 512:(qt + 1) * 512], in_=ot)

        def attention(hp, qt, fillers, fill_every=4):
            """One (head-pair, query-tile) block. ctx matmuls run one kt
            behind exp so the in-order PE stream never waits on ScalarE;
            `fillers` (list of thunks) are drained at regular kt intervals."""
            hA, hB = 2 * hp, 2 * hp + 1
            q0 = qt * 512
            ctxA = ctxp.tile([128, 512], F32, tag="ctx", name="ctxA")
            ctxB = ctxp.tile([128, 512], F32, tag="ctx", name="ctxB")
            es_tiles = {}
            fillers = list(fillers)
            nfill = len(fillers)
            fill_pts = max(1, NK // fill_every)

            def emit_ctx(kt):
                es = es_tiles.pop(kt)
                nc.tensor.matmul(
                    ctxA[0:DK + 1, :], lhsT=v_sb[:, kt, hA, :], rhs=es[:, 0:512],
                    start=(kt == 0), stop=(kt == NK - 1))
                nc.tensor.matmul(
                    ctxB[0:DK + 1, :], lhsT=v_sb[:, kt, hB, :], rhs=es[:, 512:1024],
                    start=(kt == 0), stop=(kt == NK - 1))

            fi = 0
            for kt in range(NK):
                ps = psp.tile([128, 1024], F32, tag="ps", name="pss")
                # S^T for two heads packed into PE row-groups 0-63 / 64-127
                nc.tensor.matmul(
                    ps[:, 0:512],
                    lhsT=kt_sb[0:64, hp, kt * 128:(kt + 1) * 128],
                    rhs=qt_sb[0:64, hp, q0:q0 + 512],
                    start=True, stop=True)
                nc.tensor.matmul(
                    ps[:, 512:1024],
                    lhsT=kt_sb[64:128, hp, kt * 128:(kt + 1) * 128],
                    rhs=qt_sb[64:128, hp, q0:q0 + 512],
                    start=True, stop=True, tile_position=(64, 0))
                es = expp.tile([128, 1024], BF16, tag="es", name="es")
                nc.scalar.activation(out=es, in_=ps, func=AF.Exp)
                es_tiles[kt] = es
                if kt >= 1:
                    emit_ctx(kt - 1)
                if kt % fill_every == fill_every - 1:
                    # drain an even share of the filler units
                    pt = kt // fill_every
                    upto = (pt + 1) * nfill // fill_pts
                    while fi < min(upto, nfill):
                        fillers[fi]()
                        fi += 1
            emit_ctx(NK - 1)
            while fi < nfill:
                fillers[fi]()
                fi += 1

            # Free the ctx psum banks with one fast DVE copy each; the
            # reciprocal/broadcast/multiply chain then runs SBUF-side.
            cA = stage.tile([DK + 1, 512], F32, tag="cA", name="cA")
            cB = stage.tile([DK + 1, 512], F32, tag="cB", name="cB")
            nc.vector.tensor_copy(cA, ctxA[0:DK + 1, :])
            nc.vector.tensor_copy(cB, ctxB[0:DK + 1, :])
            recA = stage.tile([1, 512], F32, tag="recA", name="recA")
            recB = stage.tile([1, 512], F32, tag="recB", name="recB")
            nc.vector.reciprocal(recA, cA[DK:DK + 1, :])
            nc.vector.reciprocal(recB, cB[DK:DK + 1, :])
            rec_dr = drp.tile([2, 512], F32, tag="rec_dr", name="rec_dr")
            nc.gpsimd.dma_start(out=rec_dr[0:1, :], in_=recA)
            nc.gpsimd.dma_start(out=rec_dr[1:2, :], in_=recB)
            bcA = stage.tile([DK, 512], F32, tag="bcA", name="bcA")
            bcB = stage.tile([DK, 512], F32, tag="bcB", name="bcB")
            nc.gpsimd.dma_start(out=bcA, in_=_bcast_ap(rec_dr[0, :], DK))
            nc.gpsimd.dma_start(out=bcB, in_=_bcast_ap(rec_dr[1, :], DK))
            for csb, bc, h in ((cA, bcA, hA), (cB, bcB, hB)):
                r0 = (h % 2) * DK
                nc.vector.tensor_tensor(
                    out=ctxT_sb[r0:r0 + DK, hp, q0:q0 + 512],
                    in0=csb[0:DK, :], in1=bc, op=ALU.mult)

        # ---------------- fused schedule (head-pair-major) ----------------
        # Upfront: all of Q[hp0] and K[hp0] + first V token tile; everything
        # else streams into attention slack as fillers.
        for half in range(4):
            qk_unit(0, 0, half)
            qk_unit(0, 1, half)
        v_unit(0)

        def mk(fn, *a):
            return lambda: fn(*a)

        # fillers per (hp, qt) block
        sched = {}
        # hp0/qt0 gets V token-tiles just-in-time (ctx(kt) needs V(tt=kt)):
        # handled specially below with fill_every=1 lookahead.
        sched[(0, 1)] = [mk(qk_unit, 1, 0, h) for h in (0, 1, 2)]
        sched[(0, 2)] = [mk(qk_unit, 1, 0, 3)] + \
                        [mk(qk_unit, 1, 1, h) for h in (0, 1)]
        sched[(0, 3)] = [mk(qk_unit, 1, 1, h) for h in (2, 3)]
        for qt in range(NQ):
            sched[(1, qt)] = [mk(qk_unit, 2, qt // 2, 2 * (qt % 2) + j)
                              for j in range(2)]
            sched[(2, qt)] = [mk(qk_unit, 3, qt // 2, 2 * (qt % 2) + j)
                              for j in range(2)]
        # hp3 runs its query tiles in order 3,0,1,2 so the last tile's O-proj
        # can pipeline early and the kernel tail is one O-proj unit set
        hp3_order = [3, 0, 1, 2]
        sched[(3, 0)] = [mk(oproj_unit, 3, et) for et in range(8)]
        sched[(3, 1)] = [mk(oproj_unit, 0, et) for et in range(8)]
        sched[(3, 2)] = [mk(oproj_unit, 1, et) for et in range(8)]

        for hp in range(HP):
            qts = hp3_order if hp == 3 else range(NQ)
            for qt in qts:
                if hp == 0 and qt == 0:
                    # V tiles arrive one kt ahead of the ctx matmul that
                    # needs them (ctx runs one kt behind S/exp)
                    attention(0, 0, [mk(v_unit, tt) for tt in range(1, TT)],
                              fill_every=1)
                else:
                    attention(hp, qt, sched.get((hp, qt), []))

        for et in range(8):
            oproj_unit(2, et)

    nc.compile()
    return nc


_PROG = None


def _get_prog() -> bass.Bass:
    global _PROG
    if _PROG is None:
        _PROG = build_program()
    return _PROG


def make_in_maps(x, Wq, bq, Wk, bk, Wv, bv, Wo, bo):
    """Build the 8 per-core input dicts from the full (unsharded) inputs."""
    bf = ml_dtypes.bfloat16
    x = np.asarray(x, np.float32)
    scale = np.float32(1.0 / np.sqrt(DK))
    WqT = np.asarray(Wq, np.float32).T * scale   # [d, e], scores scale folded in
    WkT = np.asarray(Wk, np.float32).T
    WvT = np.asarray(Wv, np.float32).T
    WoT = np.asarray(Wo, np.float32).T           # [d_in, e_out]; rows = ctx dims
    bq = np.asarray(bq, np.float32) * scale
    bk = np.asarray(bk, np.float32)
    bv = np.asarray(bv, np.float32)

    xT_b = [np.ascontiguousarray(x[b_].T).astype(bf) for b_ in range(B)]
    in_maps = []
    for c in range(N_CORES):
        b_idx, h2 = divmod(c, TP)
        sl = slice(h2 * EL, (h2 + 1) * EL)
        in_maps.append({
            "xT": xT_b[b_idx],
            "wqT": np.ascontiguousarray(WqT[:, sl]).astype(bf),
            "wkT": np.ascontiguousarray(WkT[:, sl]).astype(bf),
            "wvT": np.ascontiguousarray(WvT[:, sl]).astype(bf),
            "woT": np.ascontiguousarray(WoT[sl, :]).astype(bf),
            "bq": np.ascontiguousarray(bq[sl]),
            "bk": np.ascontiguousarray(bk[sl]),
            "bv": np.ascontiguousarray(bv[sl]),
        })
    return in_maps


def assemble_output(results, bo):
    """Sum TP partials, transpose back to [B, S, D], add output bias."""
    bo32 = np.asarray(bo, np.float32)
    out = np.empty((B, S, D), np.float32)
    for b_idx in range(B):
        acc = results[TP * b_idx]["outT"] + results[TP * b_idx + 1]["outT"]
        out[b_idx] = acc.T + bo32
    return out


def kernel(x, Wq, bq, Wk, bk, Wv, bv, Wo, bo):
    nc = _get_prog()
    in_maps = make_in_maps(x, Wq, bq, Wk, bk, Wv, bv, Wo, bo)
    res = run_bass_kernel_spmd(nc, in_maps, core_ids=list(range(N_CORES)))
    return assemble_output(res.results, bo)
